# revision 8
# baseline (speedup 1.0000x reference)
"""Trainium2 Bass kernel for quantized DenseGeneral (AQT-style int8 fake-quant matmul).

Math (must match the jax reference):
  xq = round(x / sx) * sx    sx = max(amax_row(|x|), 1e-6)/127     (per-row of x)
  kq = round(k / sk) * sk    sk = max(amax_col(|k|), 1e-6)/127     (per-col of k)
  out = xq @ kq  =  (sx (x) sk) * (x_int @ k_int)

x_int/k_int are integers in [-127, 127] — exactly representable in bf16, so the
matmul runs on the PE array in bf16 with fp32 PSUM accumulation and is exact.
Scales are applied in the epilogue (per-row via ACT scale, per-col via DVE mult).
round() uses the fp32 magic-number trick (t + 1.5*2^23 - 1.5*2^23), matching
jnp.round's round-half-to-even exactly.

Sharding over 8 cores: 4-way on flattened rows (= batch dim), 2-way on output
columns F. Per core: x [2048, 4096], k [4096, 2048] -> out [2048, 2048].

Per-core structure:
  1. kernel quant, 16 column strips of 128, single HBM read:
     strip [128p, 32dc, 128f] -> DVE strided abs-max over dc -> gpsimd
     partition_all_reduce(max) gives the per-column amax replicated on all
     partitions -> scales in registers-free tile math -> gpsimd multiply by
     127/amax -> DVE round -> resident kq [128, 32, 2048] bf16.
  2. x quant per 128-row tile: DVE abs-max reduce + scale, ACT -magic to bf16,
     store ints to DRAM scratch.
  3. matmul per 256-row superblock: XBAR-transpose-read x ints as [128d, 256m]
     tiles (batched transposes), accumulate 32 x (128x128)@(128x512) bf16
     matmuls per output tile into PSUM, epilogue applies sx (ACT) and sk (DVE).
"""

import os
import sys
from contextlib import ExitStack

import numpy as np

if "/opt/trn_rl_repo" not in sys.path and os.path.isdir("/opt/trn_rl_repo"):
    sys.path.insert(0, "/opt/trn_rl_repo")

import concourse.bass as bass
import concourse.mybir as mybir
import concourse.tile as tile
from concourse import bacc, bass_isa

# Problem geometry (hardcoded per contract)
B, S, DIM, F_FULL = 4, 2048, 4096, 4096
M_FULL = B * S              # 8192 flattened rows
N_CORES = 8
M_SHARDS, F_SHARDS = 4, 2   # core c -> (mi, fi) = divmod(c, F_SHARDS)
M = M_FULL // M_SHARDS      # 2048 rows per core
F = F_FULL // F_SHARDS      # 2048 output cols per core
P = 128
DCH = DIM // P              # 32 contraction chunks
MT = M // P                 # 16 row tiles
FS = 512                    # matmul free dim (one PSUM bank of fp32)
FT = F // FS                # 4 output column strips
KS = F // P                 # 16 kernel-quant column strips
SBROWS = 256                # matmul superblock rows (2 row tiles)
NSB = M // SBROWS           # 8 superblocks

MAGIC = float(np.float32(1.5 * 2**23))  # 12582912.0
INT8_MAX = 127.0

f32 = mybir.dt.float32
bf16 = mybir.dt.bfloat16


def build_bass(niter: int = 1):
    """niter > 1 wraps the whole body in a hardware For loop — used only for
    benchmarking (kernel time = delta(wall) / delta(niter) cancels host I/O)."""
    nc = bacc.Bacc("TRN2", target_bir_lowering=False, enable_partition_id=False)

    x_in = nc.dram_tensor("x", [M, DIM], f32, kind="ExternalInput")
    k_in = nc.dram_tensor("kern", [DIM, F], f32, kind="ExternalInput")
    out = nc.dram_tensor("out", [M, F], f32, kind="ExternalOutput")

    with tile.TileContext(nc) as tc, ExitStack() as ctx:
        if niter > 1:
            ctx.enter_context(tc.For_i(0, niter, 1))
        dram = ctx.enter_context(tc.tile_pool(name="dram", bufs=1, space="DRAM"))
        xq_dram = dram.tile([M, DIM], bf16)      # quantized x ints, natural layout

        persist = ctx.enter_context(tc.tile_pool(name="persist", bufs=1))
        kq = persist.tile([P, DCH, F], bf16)     # resident quantized kernel (128KB/part)
        sk_bcast = persist.tile([P, F], f32)     # per-col scale, replicated on partitions
        sx_all = persist.tile([P, MT], f32)      # per-row scales, col mt
        neg_magic = persist.tile([P, 1], f32)
        nc.vector.memset(neg_magic[:], -MAGIC)

        # x-quant pools stay open for the whole kernel (interleaved with matmul)
        xhp = ctx.enter_context(tc.tile_pool(name="xh", bufs=2))
        xsp = ctx.enter_context(tc.tile_pool(name="xs", bufs=4))
        xqp = ctx.enter_context(tc.tile_pool(name="xqo", bufs=2))

        def emit_xquant(mt):
            rows = slice(mt * P, (mt + 1) * P)
            xh0 = xhp.tile([P, DIM // 2], f32, tag="xh", name=f"xh0_{mt}")
            xh1 = xhp.tile([P, DIM // 2], f32, tag="xh", name=f"xh1_{mt}")
            nc.sync.dma_start(xh0[:], x_in[rows, :DIM // 2])
            nc.sync.dma_start(xh1[:], x_in[rows, DIM // 2:])
            a0 = xsp.tile([P, 1], f32, tag="ax", name=f"a0_{mt}")
            a1 = xsp.tile([P, 1], f32, tag="ax", name=f"a1_{mt}")
            nc.vector.tensor_reduce(a0[:], xh0[:], axis=mybir.AxisListType.X,
                                    op=mybir.AluOpType.max,
                                    apply_absolute_value=True)
            nc.vector.tensor_reduce(a1[:], xh1[:], axis=mybir.AxisListType.X,
                                    op=mybir.AluOpType.max,
                                    apply_absolute_value=True)
            ax = xsp.tile([P, 1], f32, tag="ax", name=f"ax_{mt}")
            nc.vector.tensor_tensor(ax[:], a0[:], a1[:], mybir.AluOpType.max)
            nc.vector.tensor_scalar_max(ax[:], ax[:], 1e-6)
            nc.vector.tensor_scalar_mul(sx_all[:, mt:mt + 1], ax[:], 1.0 / INT8_MAX)
            inv = xsp.tile([P, 1], f32, tag="ax", name=f"inv_{mt}")
            nc.vector.reciprocal(inv[:], ax[:])
            nc.vector.tensor_scalar_mul(inv[:], inv[:], INT8_MAX)
            for h, xh in ((0, xh0), (1, xh1)):
                cols = slice(h * (DIM // 2), (h + 1) * (DIM // 2))
                # t = x*inv + MAGIC (fp32, in place); ACT applies -MAGIC -> bf16
                nc.vector.tensor_scalar(xh[:], xh[:], inv[:, :1], MAGIC,
                                        mybir.AluOpType.mult,
                                        mybir.AluOpType.add)
                xqo = xqp.tile([P, DIM // 2], bf16, tag="xqo", name=f"xqo{h}_{mt}")
                nc.scalar.activation(xqo[:], xh[:],
                                     mybir.ActivationFunctionType.Identity,
                                     bias=neg_magic[:, :1])
                nc.sync.dma_start(xq_dram[rows, cols], xqo[:])

        # prime x-quant for the first superblock before the kernel-quant wave
        emit_xquant(0)
        emit_xquant(1)

        # ---------------- kernel quantization: 16 column strips, one read ----
        with tc.tile_pool(name="kqs", bufs=2) as ksp, \
             tc.tile_pool(name="kqsm", bufs=2) as ksm:
            for s in range(KS):
                cols = slice(s * P, (s + 1) * P)
                strip = ksp.tile([P, DCH, P], f32, tag="strip", name=f"kstrip{s}")
                nc.sync.dma_start(strip[:],
                                  k_in[:, cols].rearrange("(dc p) f -> p dc f", p=P))
                red = ksm.tile([P, P], f32, tag="red", name=f"red{s}")
                nc.vector.tensor_reduce(red[:],
                                        strip[:].rearrange("p dc f -> p f dc"),
                                        axis=mybir.AxisListType.X,
                                        op=mybir.AluOpType.max,
                                        apply_absolute_value=True)
                cm = ksm.tile([P, P], f32, tag="cm", name=f"cm{s}")
                nc.gpsimd.partition_all_reduce(cm[:], red[:], P,
                                               bass_isa.ReduceOp.max)
                nc.vector.tensor_scalar_max(cm[:], cm[:], 1e-6)
                nc.vector.tensor_scalar_mul(sk_bcast[:, cols], cm[:],
                                            1.0 / INT8_MAX)
                inv = ksm.tile([P, P], f32, tag="inv", name=f"kinv{s}")
                nc.vector.reciprocal(inv[:], cm[:])
                nc.vector.tensor_scalar_mul(inv[:], inv[:], INT8_MAX)
                nc.gpsimd.tensor_tensor(strip[:], strip[:],
                                        inv[:, None, :].to_broadcast((P, DCH, P)),
                                        mybir.AluOpType.mult)
                nc.vector.tensor_scalar(kq[:, :, cols], strip[:],
                                        MAGIC, -MAGIC,
                                        mybir.AluOpType.add,
                                        mybir.AluOpType.add)

        # ---------------- matmul per 256-row superblock ----------------------
        with tc.tile_pool(name="xt", bufs=5) as xtp, \
             tc.tile_pool(name="ps", bufs=8, space="PSUM") as psp, \
             tc.tile_pool(name="osb", bufs=3) as osp:
            for sb in range(NSB):
                rows = slice(sb * SBROWS, (sb + 1) * SBROWS)
                # x-quant for the NEXT superblock (stay one block ahead of PE)
                if sb < NSB - 1:
                    emit_xquant(2 * sb + 2)
                    emit_xquant(2 * sb + 3)
                # transpose-read x ints: [128d, 256m] per dc, grouped by 8 dc
                xts = []
                for g in range(4):
                    xt = xtp.tile([P, 8, SBROWS], bf16, tag="xqT",
                                  name=f"xt{sb}_{g}")
                    for k in range(8):
                        dc = g * 8 + k
                        nc.sync.dma_start_transpose(
                            xt[:, k, :], xq_dram[rows, dc * P:(dc + 1) * P])
                    xts.append(xt)
                for ml in range(SBROWS // P):
                    mt = sb * (SBROWS // P) + ml
                    mrows = slice(mt * P, (mt + 1) * P)
                    psums = [psp.tile([P, FS], f32, tag="ps", name=f"ps{mt}_{i}")
                             for i in range(FT)]
                    for dc in range(DCH):
                        g, k = divmod(dc, 8)
                        lhsT = xts[g][:, k, ml * P:(ml + 1) * P]
                        for fs in range(FT):
                            nc.tensor.matmul(psums[fs][:], lhsT,
                                             kq[:, dc, fs * FS:(fs + 1) * FS],
                                             start=(dc == 0), stop=(dc == DCH - 1))
                    for fs in range(FT):
                        osb = osp.tile([P, FS], f32, tag="osb",
                                       name=f"osb{mt}_{fs}")
                        nc.scalar.activation(osb[:], psums[fs][:],
                                             mybir.ActivationFunctionType.Copy,
                                             scale=sx_all[:, mt:mt + 1])
                        nc.vector.tensor_tensor(osb[:], osb[:],
                                                sk_bcast[:, fs * FS:(fs + 1) * FS],
                                                mybir.AluOpType.mult)
                        nc.sync.dma_start(out[mrows, fs * FS:(fs + 1) * FS], osb[:])

    nc.compile()
    return nc


_NC_CACHE = None


def _get_nc():
    global _NC_CACHE
    if _NC_CACHE is None:
        _NC_CACHE = build_bass()
    return _NC_CACHE


def make_in_maps(inputs: np.ndarray, kernel: np.ndarray):
    x = np.ascontiguousarray(np.asarray(inputs, np.float32).reshape(M_FULL, DIM))
    w = np.asarray(kernel, np.float32)
    in_maps = []
    for c in range(N_CORES):
        mi, fi = divmod(c, F_SHARDS)
        in_maps.append({
            "x": np.ascontiguousarray(x[mi * M:(mi + 1) * M]),
            "kern": np.ascontiguousarray(w[:, fi * F:(fi + 1) * F]),
        })
    return in_maps


def assemble_out(shards):
    out = np.empty((M_FULL, F_FULL), np.float32)
    for c in range(N_CORES):
        mi, fi = divmod(c, F_SHARDS)
        out[mi * M:(mi + 1) * M, fi * F:(fi + 1) * F] = shards[c]
    return out.reshape(B, S, F_FULL)


def kernel(inputs: np.ndarray, kernel: np.ndarray, _trace: bool = False):
    from concourse.bass_utils import run_bass_kernel_spmd

    nc = _get_nc()
    res = run_bass_kernel_spmd(nc, make_in_maps(inputs, kernel),
                               core_ids=list(range(N_CORES)), trace=_trace)
    out = assemble_out([r["out"] for r in res.results])
    if _trace:
        return out, res
    return out
